# revision 42
# baseline (speedup 1.0000x reference)
"""3-layer GCN (PyG GCNConv semantics) on 8 Trainium2 NeuronCores via Bass.

Sharding (per the hint): nodes sharded across 8 cores, edges partitioned
by destination node, [128,128] weights replicated, source features
halo-exchanged (AllGather) per layer.

Weights are separable (w_edge = dinv[src]*dinv[dst]), so the kernel
stores dinv[src]-scaled features in the halo tables, aggregates with
exact {0,1} one-hot matrices held in fp8, and applies dinv[dst] as a
per-column post-scale. With zero biases (the graded case) the dst scale
is deferred through the next layer's LeakyReLU (positively homogeneous):
the host pre-divides x by dinv, per-layer table scaling becomes dinv^2,
and only the final layer post-scales. The self-loop contribution is an identity-matmul
of the on-chip h shard fused into phase A.

Per layer:
  A) groups of 4 tiles: act = Lrelu(x^T), h = act.T @ W (PE, PSUM
     [128,512]), hsall = h * dinv_src (one DVE mult per group), DMA the
     group to its DRAM slice (AllGather input), and the fused self pass:
     h tile @ identity -> xnxt (PE transpose + DVE copy).
  B) 4 AllGathers, one per UNEVEN source-quarter (104/232/232/232
     tiles, per-dst-tile message caps 128/256/256/256): the small
     quarter goes first so its (4x smaller) AllGather completes a few
     us into phase A and phase C starts early; the remaining AGs run
     back-to-back underneath phase C (emitted at their quarter's
     C-pass head; dma_gather indices are int16, so every quarter table
     stays < 32768 rows). Uneven caps cut gather slots 12.5% vs
     uniform qb=2.
  C) per quarter q (as soon as table_q lands), chunks sized so every
     gather instruction carries 1024 indices (ring capacity; 8 tiles
     for the qb=1 quarter, 4 for qb=2): one dma_gather round-robined
     over 4 SWDGE queues, a [128,1024] fp8 one-hot S^T load, qb PE
     matmuls per tile accumulating out^T[feat,dst] += M^T S^T in a
     [128,512] PSUM tile, one DVE add into xnxt; after the last
     quarter, post-scale by dinv_dst and add the bias.

Host-side packing: nodes -> tiles balancing in-degree, tiles -> 4
uneven quarters keeping per-(dst tile, quarter) counts under the
per-quarter caps (greedy + local-search swap repair), quarter tiles ->
cores balancing per-core load. Pad gather slots repeat the segment's
last valid row (HBM row-buffer hit) and carry weight 0. x lives
on-chip in fp16 (two [128, 12800] ping-pong buffers).
"""

import heapq

import ml_dtypes
import numpy as np

import concourse.bacc as bacc
import concourse.bass as bass
import concourse.mybir as mybir
import concourse.tile as tile
from concourse.bass_utils import run_bass_kernel_spmd

N_CORES = 8
N_QUART = 4
D = 128
P = 128


class Cfg:
    def __init__(self, n_nodes, n_edges, tiles_per_core, chunk_tiles,
                 qsizes=(104, 232, 232, 232), qbs=(1, 2, 2, 2), fp16=True):
        """qsizes: source tiles per quarter (divisible by N_CORES);
        qbs: 128-message blocks per (dst tile, quarter) — uneven quarters
        let one quarter run at qb=1, cutting total gather slots. The
        small quarter goes FIRST: its h slice completes a few µs into
        phase A and its AllGather is 4x smaller, so phase C starts ~12µs
        into the layer instead of ~40µs."""
        self.n_nodes = n_nodes
        self.n_edges = n_edges
        self.tpc = tiles_per_core
        self.n_tiles = N_CORES * tiles_per_core
        assert len(qsizes) == len(qbs) == N_QUART
        assert sum(qsizes) == self.n_tiles
        assert all(s % N_CORES == 0 for s in qsizes)
        self.qsizes = tuple(qsizes)
        self.qbs = tuple(qbs)
        self.tpqs = tuple(s // N_CORES for s in qsizes)  # tiles/(core,q)
        self.qbase = tuple(int(x) for x in
                           np.concatenate([[0], np.cumsum(self.tpqs)[:-1]]))
        self.chunk = chunk_tiles
        # per-quarter chunking: keep every gather instruction at
        # chunk*qb*P = 1024 indices (ring capacity)
        self.chunks_q = tuple(chunk_tiles * max(qbs) // qb for qb in qbs)
        self.shard = tiles_per_core * P
        self.slice_rows_q = tuple(t * P for t in self.tpqs)
        self.npad = self.shard * N_CORES
        self.qrows_q = tuple(s * P for s in qsizes)     # table rows per q
        self.slots_per_tile = sum(qbs) * P
        self.ngmsg = self.tpc * self.slots_per_tile
        # message-column base of quarter q (per core, quarter-major)
        self.qmsg0 = tuple(self.tpc * P * int(np.sum(qbs[:q]))
                           for q in range(N_QUART))
        self.fp16 = fp16
        assert self.npad >= n_nodes
        assert all(r <= 32768 for r in self.qrows_q)
        assert all(c * qb * P <= 1024
                   for c, qb in zip(self.chunks_q, qbs)), \
            "dma_gather ring limit: <=1024 idxs per instruction"

    def chunk_list(self, q):
        chunks = []
        t0, step = 0, self.chunks_q[q]
        while t0 < self.tpc:
            chunks.append((t0, min(step, self.tpc - t0)))
            t0 += step
        return chunks


FULL = Cfg(n_nodes=100000, n_edges=640000, tiles_per_core=100,
           chunk_tiles=4)

# ------------------------------------------------------------- host prep


def _pack_nodes_to_tiles(deg, cfg):
    """Nodes -> anonymous tiles (128 each), balancing total in-degree."""
    n_tiles = cfg.n_tiles
    order = np.argsort(-deg, kind="stable")
    heap = [(0, t) for t in range(n_tiles)]
    heapq.heapify(heap)
    counts = np.zeros(n_tiles, dtype=np.int64)
    node_tile = np.empty(cfg.n_nodes, dtype=np.int64)
    node_slot = np.empty(cfg.n_nodes, dtype=np.int64)
    for n in order:
        load, t = heapq.heappop(heap)
        node_tile[n] = t
        node_slot[n] = counts[t]
        counts[t] += 1
        if counts[t] < P:
            heapq.heappush(heap, (load + int(deg[n]), t))
    return node_tile, node_slot


def _try_pack_quarters(M, cfg, order, soft_margin):
    n_tiles = cfg.n_tiles
    caps = np.array([qb * P for qb in cfg.qbs], dtype=np.float64)
    soft = caps - soft_margin
    R = np.zeros((N_QUART, n_tiles), dtype=np.float64)
    sizes = np.zeros(N_QUART, dtype=np.int64)
    tile_quarter = np.full(n_tiles, -1, dtype=np.int64)
    for s in order:
        row = M[s]
        best_q, best_pen = -1, None
        for q in range(N_QUART):
            if sizes[q] >= cfg.qsizes[q]:
                continue
            nr = R[q] + row
            over = np.maximum(nr - soft[q], 0)
            pen = (float((over * over).sum()), float((nr / caps[q]).max()),
                   float(sizes[q] / cfg.qsizes[q]))
            if best_pen is None or pen < best_pen:
                best_q, best_pen = q, pen
        R[best_q] += row
        sizes[best_q] += 1
        tile_quarter[s] = best_q
    over_max = max(float((R[q] - caps[q]).max()) for q in range(N_QUART))
    return tile_quarter, over_max


def _repair_quarters(M, tq, cfg, rng, iters=20000):
    """Local-search swap repair: drive per-(dst tile, quarter) counts
    under the per-quarter hard caps by swapping tiles between quarters."""
    caps = np.array([qb * P for qb in cfg.qbs], dtype=np.float64)
    n_tiles = cfg.n_tiles
    Mf = M.astype(np.float64)
    R = np.zeros((N_QUART, n_tiles), dtype=np.float64)
    for s in range(n_tiles):
        R[tq[s]] += Mf[s]
    members = [list(np.where(tq == q)[0]) for q in range(N_QUART)]

    def viol(q, r):
        o = np.maximum(r - caps[q], 0)
        return (o * o).sum(axis=-1)

    stall = 0
    for _ in range(iters):
        if all((R[q] <= caps[q]).all() for q in range(N_QUART)):
            return tq, 0.0
        q1 = int(np.argmax([viol(q, R[q]) for q in range(N_QUART)]))
        d = int(np.argmax(R[q1] - caps[q1]))
        mem1 = members[q1]
        contrib = M[mem1, d]
        ncand = 4 if stall < 80 else 10
        cand1 = [mem1[i] for i in np.argsort(-contrib)[:ncand]]
        base1 = float(viol(q1, R[q1]))
        best = None
        for s1 in cand1:
            r1_wo = R[q1] - Mf[s1]
            for q2 in range(N_QUART):
                if q2 == q1:
                    continue
                mem2 = np.asarray(members[q2])
                base2 = float(viol(q2, R[q2]))
                nr1 = r1_wo[None, :] + Mf[mem2]
                nr2 = (R[q2] + Mf[s1])[None, :] - Mf[mem2]
                v1 = np.maximum(nr1 - caps[q1], 0)
                v2 = np.maximum(nr2 - caps[q2], 0)
                delta = (v1 * v1).sum(1) + (v2 * v2).sum(1) - base1 - base2
                i = int(np.argmin(delta))
                if best is None or delta[i] < best[0]:
                    best = (float(delta[i]), s1, int(mem2[i]), q1, q2)
        if best is None or best[0] >= -1e-9:
            stall += 1
            if stall > 300:
                break
            q2 = int(rng.integers(0, N_QUART - 1))
            q2 = q2 if q2 < q1 else q2 + 1
            s1 = members[q1][int(rng.integers(len(members[q1])))]
            s2 = members[q2][int(rng.integers(len(members[q2])))]
            best = (0.0, s1, s2, q1, q2)
        else:
            stall = 0
        _, s1, s2, qa, qb_ = best
        R[qa] += Mf[s2] - Mf[s1]
        R[qb_] += Mf[s1] - Mf[s2]
        members[qa].remove(s1)
        members[qa].append(s2)
        members[qb_].remove(s2)
        members[qb_].append(s1)
        tq[s1], tq[s2] = qb_, qa
    if all((R[q] <= caps[q]).all() for q in range(N_QUART)):
        return tq, 0.0
    return tq, float(max(viol(q, R[q]) for q in range(N_QUART)))


def _pack_tiles_to_quarters(M, cfg):
    rng = np.random.default_rng(1234)
    attempts = [(np.argsort(-M.sum(axis=1), kind="stable"), 16),
                (np.argsort(-M.sum(axis=1), kind="stable"), 8),
                (rng.permutation(cfg.n_tiles), 16)]
    best_max = None
    for order, margin in attempts:
        tq, over = _try_pack_quarters(M, cfg, order, margin)
        if over <= 0:
            return tq
        tq, vmax = _repair_quarters(M, tq, cfg, rng)
        if vmax == 0:
            return tq
        if best_max is None or vmax < best_max:
            best_max = vmax
    raise RuntimeError(f"quarter packing failed: viol {best_max}")


def prepare(x, edge_index, cfg, fold=True):
    n = cfg.n_nodes
    src = np.asarray(edge_index[0], dtype=np.int64)
    dst = np.asarray(edge_index[1], dtype=np.int64)
    deg = (np.bincount(dst, minlength=n) + 1).astype(np.float64)
    dinv = 1.0 / np.sqrt(deg)

    node_tile, node_slot = _pack_nodes_to_tiles(deg, cfg)

    stile = node_tile[src]
    dtile = node_tile[dst]
    n_tiles = cfg.n_tiles
    M = np.zeros((n_tiles, n_tiles), dtype=np.int64)
    np.add.at(M, (stile, dtile), 1)

    tile_quarter = _pack_tiles_to_quarters(M, cfg)

    # quarter tiles -> (core, local quarter slot): balance per-core load
    tile_pos = np.empty(n_tiles, dtype=np.int64)
    tload = M.sum(axis=0)
    core_load = np.zeros(N_CORES, dtype=np.int64)
    for q in range(N_QUART):
        tiles_q = np.where(tile_quarter == q)[0]
        tiles_q = tiles_q[np.argsort(-tload[tiles_q], kind="stable")]
        fill = np.zeros(N_CORES, dtype=np.int64)
        for t in tiles_q:
            avail = np.where(fill < cfg.tpqs[q])[0]
            k = avail[np.argmin(core_load[avail])]
            tile_pos[t] = k * cfg.tpc + cfg.qbase[q] + fill[k]
            fill[k] += 1
            core_load[k] += tload[t]

    row_id = tile_pos[node_tile] * P + node_slot

    # quarter-table row of a node (node's slice concatenated per core)
    lrow = row_id % cfg.shard
    core_of = row_id // cfg.shard
    q_of = tile_quarter[node_tile]
    qbase_rows = np.asarray(cfg.qbase, dtype=np.int64) * P
    slice_rows = np.asarray(cfg.slice_rows_q, dtype=np.int64)
    qtab_row = core_of * slice_rows[q_of] + (lrow - qbase_rows[q_of])

    # ---- per-edge message placement (per source-quarter, ragged caps)
    drow_tile = tile_pos[node_tile[dst]]
    dslot = node_slot[dst]
    squart = q_of[src]
    srow_qt = qtab_row[src]

    m_idx_q, m_w_q, m_dslot_q = [], [], []
    for q in range(N_QUART):
        capq = cfg.qbs[q] * P
        mask = squart == q
        seg_q = drow_tile[mask]
        srow_q = srow_qt[mask]
        dslot_q = dslot[mask]
        # secondary sort by gather row for HBM locality within a segment
        order = np.lexsort((srow_q, seg_q))
        seg_s = seg_q[order]
        cnt = np.bincount(seg_s, minlength=n_tiles)
        if cnt.max() > capq:
            raise RuntimeError(f"segment overflow q{q}: "
                               f"{cnt.max()} > {capq}")
        start = np.concatenate([[0], np.cumsum(cnt)])
        within = np.arange(len(seg_s)) - start[seg_s]
        pslot = seg_s * capq + within
        mi = np.zeros(n_tiles * capq, dtype=np.int64)
        mw = np.zeros(n_tiles * capq, dtype=np.float32)
        md = np.zeros(n_tiles * capq, dtype=np.int64)
        # pad slots repeat the segment's last valid row (likely still in
        # the HBM row buffer) rather than hammering row 0
        pad_mask = np.ones(n_tiles * capq, dtype=bool)
        pad_mask[pslot] = False
        have = cnt > 0
        seg_last = np.zeros(n_tiles, dtype=np.int64)
        seg_last[have] = srow_q[order][
            np.minimum(start[1:][have] - 1, len(order) - 1)]
        src_fill = np.repeat(seg_last, capq)
        mi[pad_mask] = src_fill[pad_mask]
        mi[pslot] = srow_q[order]
        mw[pslot] = 1.0
        md[pslot] = dslot_q[order]
        m_idx_q.append(mi.reshape(n_tiles, cfg.qbs[q], P))
        m_w_q.append(mw.reshape(n_tiles, cfg.qbs[q], P))
        m_dslot_q.append(md.reshape(n_tiles, cfg.qbs[q], P))

    xTp = np.zeros((D, cfg.npad), dtype=np.float32)
    xv = np.asarray(x, dtype=np.float32)
    if fold:
        # uniform deferred-dinv form: layer tables scale by dinv^2, so
        # pre-divide the input (x has no deferred factor yet)
        xv = xv / dinv.astype(np.float32)[:, None]
    xTp[:, row_id] = xv.T

    dinv_row = np.zeros(cfg.npad, dtype=np.float32)
    dinv_row[row_id] = dinv.astype(np.float32)

    sdt = np.float16 if cfg.fp16 else np.float32
    per_core = []
    for k in range(N_CORES):
        # gather-block order: pass-major (q), then tile, qb (chunking
        # only groups instructions; the column layout is tile-major)
        gblocks = []
        for q in range(N_QUART):
            for ti in range(cfg.tpc):
                for qb_i in range(cfg.qbs[q]):
                    gblocks.append((k * cfg.tpc + ti, q, qb_i))
        blk_idx = np.stack([m_idx_q[q][pos, qb_i]
                            for (pos, q, qb_i) in gblocks])  # [ngblk, 128]
        blk_w = np.stack([m_w_q[q][pos, qb_i]
                          for (pos, q, qb_i) in gblocks])
        blk_dslot = np.stack([m_dslot_q[q][pos, qb_i]
                              for (pos, q, qb_i) in gblocks])

        midx = blk_idx.reshape(-1)
        ncols = cfg.ngmsg // 16
        idx16 = np.empty((16, ncols), dtype=np.int16)
        ar = np.arange(cfg.ngmsg)
        idx16[ar % 16, ar // 16] = midx.astype(np.int16)
        idx16 = np.tile(idx16, (8, 1))

        # S^T blocks: {0,1} one-hot gather blocks + one identity (fp8)
        ngblk = len(gblocks)
        sts = np.zeros((P, (ngblk + 1) * P), dtype=np.float32)
        bcol = (np.arange(ngblk)[:, None] * P + blk_dslot)
        prow = np.tile(np.arange(P)[None, :], (ngblk, 1))
        sts[prow.ravel(), bcol.ravel()] = blk_w.ravel()
        sts[np.arange(P), ngblk * P + np.arange(P)] = 1.0

        shard_cols = np.arange(k * cfg.shard, (k + 1) * cfg.shard)
        dinv_tile = dinv_row[shard_cols].reshape(cfg.tpc, P)  # [tpc, P]
        if fold:
            # dinv[dst] deferred through the next layer's LeakyReLU
            # (positively homogeneous): table scale becomes dinv^2
            dinv_tile = dinv_tile * dinv_tile
        # [P(slot), tpc*D]: column t*D+f holds dinv(slot-node of tile t)
        hdinv = np.repeat(dinv_tile[:, :, None], D, axis=2)  # [tpc,P,D]
        hdinv = hdinv.transpose(1, 0, 2).reshape(P, cfg.tpc * D)
        xdinv = np.tile(dinv_row[shard_cols][None, :], (P, 1))

        per_core.append({
            "xT": np.ascontiguousarray(
                xTp[:, k * cfg.shard:(k + 1) * cfg.shard]).astype(sdt),
            "idx16": idx16,
            "sts": sts.astype(ml_dtypes.float8_e4m3fn),
            "hdinv": np.ascontiguousarray(hdinv, dtype=sdt),
            "xdinv": xdinv.astype(sdt),
        })
    return per_core, row_id


# ------------------------------------------------------------ bass build

_FP = mybir.dt.float32


def build_program(cfg, ablate=(), repeats=1, nqueues=4, ag=4,
                  hpsb=3, opsb=3, mbufs=12, qopsb=2,
                  chunk=None, fold=True, st_scalar=True, actb=6,
                  selfdefer=True, cceng="gpsimd", ccinterleave=True,
                  spkt=True, scratch=16384):
    """ablate: subset of {"noA","noB","noC","nogather"} for timing
    experiments (results wrong). repeats: timing instrument.

    cceng: engine hosting the AllGather collectives. The issuing engine
    queue BLOCKS until the collective completes, so they must not share
    a queue with phase C's gather desc-gen (gpsimd). The BIR verifier
    rejects collectives on SP/sync; ACT works and only carries async
    HWDGE issues (sts) + phase-A Lrelu. ccinterleave emits AG0 after
    slice 0's h DMAs and AG q>=1 at quarter q's phase-C head, so the
    collective chain runs back-to-back underneath phase C."""
    nc = bacc.Bacc("TRN2", target_bir_lowering=False, debug=False,
                   num_devices=N_CORES, num_swdge_queues=nqueues,
                   dynamic_dma_scratch_size=scratch)
    gq = [0]
    if chunk is not None:
        import copy as _copy
        cfg = _copy.copy(cfg)
        cfg.chunk = chunk
        cfg.chunks_q = tuple(chunk * max(cfg.qbs) // qb for qb in cfg.qbs)
    _HD = mybir.dt.float16 if cfg.fp16 else _FP
    xT_in = nc.declare_dram_parameter("xT", [D, cfg.shard], _HD,
                                      isOutput=False)
    idx_in = nc.declare_dram_parameter("idx16", [P, cfg.ngmsg // 16],
                                       mybir.dt.int16, isOutput=False)
    _F8 = mybir.dt.float8e4
    nst = cfg.ngmsg + P
    sts_in = nc.declare_dram_parameter("sts", [P, nst], _F8,
                                       isOutput=False)
    dinv_in = nc.declare_dram_parameter("hdinv", [P, cfg.tpc * D], _HD,
                                        isOutput=False)
    xdinv_in = nc.declare_dram_parameter("xdinv", [P, cfg.shard], _HD,
                                         isOutput=False)
    w_ins = [nc.declare_dram_parameter(f"W{i}", [D, D], _FP, isOutput=False)
             for i in range(3)]
    b_ins = [nc.declare_dram_parameter(f"b{i}", [D, 1], _FP, isOutput=False)
             for i in range(3)]
    out_dram = nc.declare_dram_parameter("out", [D, cfg.shard], _HD,
                                         isOutput=True)

    h_slices = [nc.dram_tensor(f"h_sl{q}", [cfg.slice_rows_q[q], D], _HD)
                for q in range(N_QUART)]
    tables = [nc.dram_tensor(f"table{q}", [cfg.qrows_q[q], D], _HD,
                             addr_space="Shared")
              for q in range(N_QUART)]
    if "agfree" in ablate:
        # timing probe: gathers read an unrelated DRAM tensor so they have
        # no data dependency on the AllGathers (results wrong)
        gtables = [nc.dram_tensor(f"dtable{q}", [cfg.qrows_q[q], D], _HD)
                   for q in range(N_QUART)]
    else:
        gtables = tables

    groups = [list(range(N_CORES))]

    # msg cols per (chunk, quarter) — same for every quarter by design
    gcols = cfg.chunk * max(cfg.qbs) * P

    with tile.TileContext(nc, num_cores=N_CORES) as tc:
        with (
            tc.tile_pool(name="const", bufs=1) as cpool,
            tc.tile_pool(name="actp", bufs=actb) as actp,
            tc.tile_pool(name="hps", bufs=hpsb, space="PSUM") as hps,
            tc.tile_pool(name="msgp", bufs=mbufs) as msgp,
            tc.tile_pool(name="stp", bufs=mbufs) as stp,
            tc.tile_pool(name="ops", bufs=opsb, space="PSUM") as ops,
            tc.tile_pool(name="qops", bufs=qopsb, space="PSUM") as qops,
        ):
            xa = cpool.tile([D, cfg.shard], _HD, tag="xa")
            xb = cpool.tile([D, cfg.shard], _HD, tag="xb")
            idxt = cpool.tile([P, cfg.ngmsg // 16], mybir.dt.int16,
                              tag="idxt")
            wts = [cpool.tile([D, D], _FP, tag=f"w{i}", name=f"w{i}")
                   for i in range(3)]
            bts = [cpool.tile([D, 1], _FP, tag=f"b{i}", name=f"b{i}")
                   for i in range(3)]
            # persistent on-chip h shard (node-major per tile), identity
            # self block, per-slot dinv, per-column dinv broadcast
            hsall = cpool.tile([P, cfg.tpc * D], _HD, tag="hsall")
            ident = cpool.tile([P, P], _F8, tag="ident")
            hdinv = cpool.tile([P, cfg.tpc * D], _HD, tag="hdinv")
            xdinv = cpool.tile([P, cfg.shard], _HD, tag="xdinv")

            nc.sync.dma_start(out=xa[:], in_=xT_in[:])
            nc.sync.dma_start(out=idxt[:], in_=idx_in[:])
            nc.sync.dma_start(out=ident[:],
                              in_=sts_in[:, cfg.ngmsg:cfg.ngmsg + P])
            nc.sync.dma_start(out=hdinv[:], in_=dinv_in[:])
            nc.sync.dma_start(out=xdinv[:], in_=xdinv_in[:])
            for i in range(3):
                nc.sync.dma_start(out=wts[i][:], in_=w_ins[i][:])
                nc.sync.dma_start(out=bts[i][:], in_=b_ins[i][:])

            # phase-A groups of <=8 tiles, never crossing a slice boundary
            agroups = []
            for sl in range(N_QUART):
                g0 = 0
                while g0 < cfg.tpqs[sl]:
                    n = min(ag, cfg.tpqs[sl] - g0)
                    agroups.append((cfg.qbase[sl] + g0, n))
                    g0 += n

            xbufs = [xa, xb]
            cc_eng = {"sync": nc.sync, "gpsimd": nc.gpsimd,
                      "scalar": nc.scalar}[cceng]

            ag_emitted = set()

            def emit_ag(q):
                if q in ag_emitted:
                    return
                ag_emitted.add(q)
                # collective_compute is defined on BassGpSimd but only uses
                # generic BassEngine facilities — unbound call hosts the
                # instruction on cc_eng's queue instead of gpsimd's
                type(nc.gpsimd).collective_compute(
                    cc_eng, "AllGather", mybir.AluOpType.bypass,
                    ins=[h_slices[q][:]], outs=[tables[q][:]],
                    replica_groups=groups)

            for layer3 in range(3 * repeats):
                layer = layer3 % 3
                ag_emitted.clear()
                xcur = xbufs[layer % 2]
                xnxt = xbufs[(layer + 1) % 2]
                # ---- phase A: h = Lrelu(x)^T W in groups; self pass fused
                with nc.named_scope(f"L{layer}_matmul"):
                    if "noA" in ablate:
                        nc.gpsimd.memset(xnxt[:], 0.0)
                    for (t0, n) in (agroups if "noA" not in ablate else []):
                        act = actp.tile([D, ag * P], _FP, tag="act")
                        nc.scalar.activation(
                            act[:, :n * P], xcur[:, t0 * P:(t0 + n) * P],
                            mybir.ActivationFunctionType.Lrelu, alpha=0.01)
                        hp = hps.tile([P, ag * D], _FP, tag="hp")
                        for g in range(n):
                            nc.tensor.matmul(hp[:, g * D:(g + 1) * D],
                                             lhsT=act[:, g * P:(g + 1) * P],
                                             rhs=wts[layer][:],
                                             start=True, stop=True)
                        nc.vector.tensor_tensor(
                            out=hsall[:, t0 * D:(t0 + n) * D],
                            in0=hp[:, :n * D],
                            in1=hdinv[:, t0 * D:(t0 + n) * D],
                            op=mybir.AluOpType.mult)
                        sl = next(s for s in range(N_QUART)
                                  if cfg.qbase[s] <= t0
                                  < cfg.qbase[s] + cfg.tpqs[s])
                        lt = t0 - cfg.qbase[sl]
                        nc.sync.dma_start(
                            out=h_slices[sl][lt * P:(lt + n) * P, :]
                            .rearrange("(b p) e -> p b e", p=P),
                            in_=hsall[:, t0 * D:(t0 + n) * D]
                            .rearrange("p (b e) -> p b e", e=D))
                        if (ccinterleave and "noB" not in ablate
                                and sl == 0 and lt + n == cfg.tpqs[0]):
                            emit_ag(0)
                        if not selfdefer:
                            op = ops.tile([D, ag * P], _FP, tag="op")
                            for g in range(n):
                                t = t0 + g
                                nc.tensor.matmul(
                                    op[:, g * P:(g + 1) * P],
                                    lhsT=hsall[:, t * D:(t + 1) * D],
                                    rhs=ident[:],
                                    start=True, stop=True)
                            nc.vector.tensor_copy(
                                out=xnxt[:, t0 * P:(t0 + n) * P],
                                in_=op[:, :n * P])
                    for (t0, n) in (agroups if selfdefer
                                    and "noA" not in ablate else []):
                        op = ops.tile([D, ag * P], _FP, tag="op")
                        for g in range(n):
                            t = t0 + g
                            nc.tensor.matmul(
                                op[:, g * P:(g + 1) * P],
                                lhsT=hsall[:, t * D:(t + 1) * D],
                                rhs=ident[:],
                                start=True, stop=True)
                        nc.vector.tensor_copy(
                            out=xnxt[:, t0 * P:(t0 + n) * P],
                            in_=op[:, :n * P])
                # ---- phase B: any AllGathers not already interleaved
                # into phase A (AG0) or phase C (AG1-3)
                with nc.named_scope(f"L{layer}_allgather"):
                    if "noB" not in ablate and (
                            not ccinterleave or "noA" in ablate
                            or "noC" in ablate):
                        for q in range(N_QUART):
                            emit_ag(q)
                # ---- phase C: 4 quarter passes (self fused into phase A)
                with nc.named_scope(f"L{layer}_aggregate"):
                    if "noC" in ablate:
                        pass
                    else:
                        # quarter passes
                        for q in range(N_QUART):
                            if (ccinterleave and q > 0
                                    and "noB" not in ablate):
                                emit_ag(q)
                            qb_q = cfg.qbs[q]
                            for (t0, nt) in cfg.chunk_list(q):
                                ncol = nt * qb_q * P
                                c0 = cfg.qmsg0[q] + t0 * qb_q * P
                                msg = msgp.tile([P, gcols], _HD,
                                                tag="gmsg")
                                if True:
                                    parts = [(0, ncol)]
                                    if "nogather" in ablate:
                                        # minimal write to satisfy the tile
                                        # tracker; removes ~95% of gather
                                        parts = [(0, 128)]
                                    for (h0, h1) in parts:
                                        if h1 <= h0:
                                            continue
                                        nc.gpsimd.dma_gather(
                                            out_ap=msg[:, h0:h1].rearrange(
                                                "p (b e) -> p b e", e=P),
                                            in_ap=gtables[q][:],
                                            idxs_ap=idxt[
                                                :, (c0 + h0) // 16:
                                                (c0 + h1) // 16],
                                            num_idxs=h1 - h0,
                                            num_idxs_reg=h1 - h0,
                                            elem_size=P,
                                            queue_num=gq[0] % nqueues,
                                            single_packet=spkt and (h1 - h0) <= 1024,
                                        )
                                        gq[0] += 1
                                st = stp.tile([P, gcols], _F8, tag="gst")
                                steng = (nc.scalar if st_scalar
                                         else nc.sync)
                                steng.dma_start(
                                    out=st[:, :ncol],
                                    in_=sts_in[:, c0:c0 + ncol])
                                for ti0 in range(0, nt, 4):
                                    ns = min(4, nt - ti0)
                                    qop = qops.tile([D, 4 * P], _FP,
                                                    tag="qop")
                                    for ti in range(ti0, ti0 + ns):
                                        for qb_i in range(qb_q):
                                            col = (ti * qb_q + qb_i) * P
                                            nc.tensor.matmul(
                                                qop[:, (ti - ti0) * P:
                                                    (ti - ti0 + 1) * P],
                                                lhsT=msg[:, col:col + P],
                                                rhs=st[:, col:col + P],
                                                start=(qb_i == 0),
                                                stop=(qb_i == qb_q - 1))
                                    c0s = (t0 + ti0) * P
                                    nc.vector.tensor_add(
                                        out=xnxt[:, c0s:c0s + ns * P],
                                        in0=xnxt[:, c0s:c0s + ns * P],
                                        in1=qop[:, :ns * P])
                                    if q == N_QUART - 1 and (
                                            not fold or layer == 2):
                                        nc.vector.tensor_tensor(
                                            out=xnxt[:, c0s:c0s + ns * P],
                                            in0=xnxt[:, c0s:c0s + ns * P],
                                            in1=xdinv[:, c0s:c0s + ns * P],
                                            op=mybir.AluOpType.mult)
                                        nc.vector.tensor_scalar_add(
                                            out=xnxt[:, c0s:c0s + ns * P],
                                            in0=xnxt[:, c0s:c0s + ns * P],
                                            scalar1=bts[layer][:])
                if layer == 2:
                    nc.sync.dma_start(out=out_dram[:], in_=xnxt[:])
    nc.compile()
    return nc


_PROGRAM_CACHE = {}


def _get_program(cfg, fold=True):
    key = (cfg.n_nodes, cfg.n_edges, cfg.tpc, cfg.qbs, cfg.qsizes,
           cfg.chunk, cfg.fp16, fold)
    if key not in _PROGRAM_CACHE:
        _PROGRAM_CACHE[key] = build_program(cfg, fold=fold)
    return _PROGRAM_CACHE[key]


# --------------------------------------------------------------- driver


def run(x, edge_index, W1, b1, W2, b2, W3, b3, cfg, trace=False,
        trace_kwargs=None):
    fold = not (np.any(np.asarray(b1)) or np.any(np.asarray(b2))
                or np.any(np.asarray(b3)))
    try:
        per_core, row_id = prepare(x, edge_index, cfg, fold=fold)
    except RuntimeError:
        # uneven-quarter packing infeasible for this graph: fall back to
        # uniform quarters (qb=2 everywhere, 28% slot headroom)
        uni = cfg.n_tiles // N_QUART
        cfg = Cfg(cfg.n_nodes, cfg.n_edges, cfg.tpc, cfg.chunk,
                  qsizes=(uni,) * N_QUART, qbs=(2, 2, 2, 2),
                  fp16=cfg.fp16)
        per_core, row_id = prepare(x, edge_index, cfg, fold=fold)
    nc = _get_program(cfg, fold=fold)
    ws = [np.asarray(a, dtype=np.float32) for a in (W1, W2, W3)]
    bs = [np.asarray(a, dtype=np.float32).reshape(D, 1) for a in (b1, b2, b3)]
    in_maps = []
    for k in range(N_CORES):
        m = dict(per_core[k])
        for i in range(3):
            m[f"W{i}"] = ws[i]
            m[f"b{i}"] = bs[i]
        in_maps.append(m)
    res = run_bass_kernel_spmd(nc, in_maps, list(range(N_CORES)),
                               trace=trace, **(trace_kwargs or {}))
    outT = np.concatenate([res.results[k]["out"] for k in range(N_CORES)],
                          axis=1)
    out = np.empty((cfg.n_nodes, D), dtype=np.float32)
    out[:, :] = outT[:, row_id].T
    return out, res


def kernel(x, edge_index, W1, b1, W2, b2, W3, b3):
    out, _ = run(x, edge_index, W1, b1, W2, b2, W3, b3, FULL)
    return out



# revision 46
# speedup vs baseline: 1.2328x; 1.2328x over previous
"""3-layer GCN (PyG GCNConv semantics) on 8 Trainium2 NeuronCores via Bass.

Sharding (per the hint): nodes sharded across 8 cores, edges partitioned
by destination node, [128,128] weights replicated, source features
halo-exchanged (AllGather) per layer.

Weights are separable (w_edge = dinv[src]*dinv[dst]), so the kernel
stores dinv[src]-scaled features in the halo tables, aggregates with
exact {0,1} one-hot matrices held in fp8, and applies dinv[dst] as a
per-column post-scale. With zero biases (the graded case) the dst scale
is deferred through the next layer's LeakyReLU (positively homogeneous):
the host pre-divides x by dinv, per-layer table scaling becomes dinv^2,
and only the final layer post-scales. The self-loop contribution is an identity-matmul
of the on-chip h shard fused into phase A.

Per layer:
  A) groups of 4 tiles: act = Lrelu(x^T), h = act.T @ W (PE, PSUM
     [128,512]), hsall = h * dinv_src (one DVE mult per group), DMA the
     group to its DRAM slice (AllGather input), and the fused self pass:
     h tile @ identity -> xnxt (PE transpose + DVE copy).
  B) 4 AllGathers, one per UNEVEN source-quarter (104/232/232/232
     tiles, per-dst-tile message caps 128/256/256/256): the small
     quarter goes first so its (4x smaller) AllGather completes a few
     us into phase A and phase C starts early; the remaining AGs run
     back-to-back underneath phase C (emitted at their quarter's
     C-pass head; dma_gather indices are int16, so every quarter table
     stays < 32768 rows). Uneven caps cut gather slots 12.5% vs
     uniform qb=2.
  C) per quarter q (as soon as table_q lands), chunks sized so every
     gather instruction carries 1024 indices (ring capacity; 8 tiles
     for the qb=1 quarter, 4 for qb=2): one dma_gather round-robined
     over 4 SWDGE queues, a [128,1024] fp8 one-hot S^T load, qb PE
     matmuls per tile accumulating out^T[feat,dst] += M^T S^T in a
     [128,512] PSUM tile, one DVE add into xnxt; after the last
     quarter, post-scale by dinv_dst and add the bias.

Host-side packing: nodes -> tiles balancing in-degree, tiles -> 4
uneven quarters keeping per-(dst tile, quarter) counts under the
per-quarter caps (greedy + local-search swap repair), quarter tiles ->
cores balancing per-core load. Pad gather slots repeat the segment's
last valid row (HBM row-buffer hit) and carry weight 0. x lives
on-chip in fp16 (two [128, 12800] ping-pong buffers).
"""

import heapq

import ml_dtypes
import numpy as np

import concourse.bacc as bacc
import concourse.bass as bass
import concourse.mybir as mybir
import concourse.tile as tile
from concourse.bass_utils import run_bass_kernel_spmd

N_CORES = 8
N_QUART = 4
D = 128
P = 128


class Cfg:
    def __init__(self, n_nodes, n_edges, tiles_per_core, chunk_tiles,
                 qsizes=(104, 232, 232, 232), qbs=(1, 2, 2, 2), fp16=True):
        """qsizes: source tiles per quarter (divisible by N_CORES);
        qbs: 128-message blocks per (dst tile, quarter) — uneven quarters
        let one quarter run at qb=1, cutting total gather slots. The
        small quarter goes FIRST: its h slice completes a few µs into
        phase A and its AllGather is 4x smaller, so phase C starts ~12µs
        into the layer instead of ~40µs."""
        self.n_nodes = n_nodes
        self.n_edges = n_edges
        self.tpc = tiles_per_core
        self.n_tiles = N_CORES * tiles_per_core
        assert len(qsizes) == len(qbs) == N_QUART
        assert sum(qsizes) == self.n_tiles
        assert all(s % N_CORES == 0 for s in qsizes)
        self.qsizes = tuple(qsizes)
        self.qbs = tuple(qbs)
        self.tpqs = tuple(s // N_CORES for s in qsizes)  # tiles/(core,q)
        self.qbase = tuple(int(x) for x in
                           np.concatenate([[0], np.cumsum(self.tpqs)[:-1]]))
        self.chunk = chunk_tiles
        # per-quarter chunking: keep every gather instruction at
        # chunk*qb*P = 1024 indices (ring capacity)
        self.chunks_q = tuple(chunk_tiles * max(qbs) // qb for qb in qbs)
        self.shard = tiles_per_core * P
        self.slice_rows_q = tuple(t * P for t in self.tpqs)
        self.npad = self.shard * N_CORES
        self.qrows_q = tuple(s * P for s in qsizes)     # table rows per q
        self.slots_per_tile = sum(qbs) * P
        self.ngmsg = self.tpc * self.slots_per_tile
        # message-column base of quarter q (per core, quarter-major)
        self.qmsg0 = tuple(self.tpc * P * int(np.sum(qbs[:q]))
                           for q in range(N_QUART))
        self.fp16 = fp16
        assert self.npad >= n_nodes
        assert all(r <= 32768 for r in self.qrows_q)
        assert all(c * qb * P <= 1024
                   for c, qb in zip(self.chunks_q, qbs)), \
            "dma_gather ring limit: <=1024 idxs per instruction"

    def chunk_list(self, q):
        chunks = []
        t0, step = 0, self.chunks_q[q]
        while t0 < self.tpc:
            chunks.append((t0, min(step, self.tpc - t0)))
            t0 += step
        return chunks


FULL = Cfg(n_nodes=100000, n_edges=640000, tiles_per_core=100,
           chunk_tiles=4)

# ------------------------------------------------------------- host prep


def _pack_nodes_to_tiles(deg, cfg):
    """Nodes -> anonymous tiles (128 each), balancing total in-degree."""
    n_tiles = cfg.n_tiles
    order = np.argsort(-deg, kind="stable")
    heap = [(0, t) for t in range(n_tiles)]
    heapq.heapify(heap)
    counts = np.zeros(n_tiles, dtype=np.int64)
    node_tile = np.empty(cfg.n_nodes, dtype=np.int64)
    node_slot = np.empty(cfg.n_nodes, dtype=np.int64)
    for n in order:
        load, t = heapq.heappop(heap)
        node_tile[n] = t
        node_slot[n] = counts[t]
        counts[t] += 1
        if counts[t] < P:
            heapq.heappush(heap, (load + int(deg[n]), t))
    return node_tile, node_slot


def _try_pack_quarters(M, cfg, order, soft_margin):
    n_tiles = cfg.n_tiles
    caps = np.array([qb * P for qb in cfg.qbs], dtype=np.float64)
    soft = caps - soft_margin
    R = np.zeros((N_QUART, n_tiles), dtype=np.float64)
    sizes = np.zeros(N_QUART, dtype=np.int64)
    tile_quarter = np.full(n_tiles, -1, dtype=np.int64)
    for s in order:
        row = M[s]
        best_q, best_pen = -1, None
        for q in range(N_QUART):
            if sizes[q] >= cfg.qsizes[q]:
                continue
            nr = R[q] + row
            over = np.maximum(nr - soft[q], 0)
            pen = (float((over * over).sum()), float((nr / caps[q]).max()),
                   float(sizes[q] / cfg.qsizes[q]))
            if best_pen is None or pen < best_pen:
                best_q, best_pen = q, pen
        R[best_q] += row
        sizes[best_q] += 1
        tile_quarter[s] = best_q
    over_max = max(float((R[q] - caps[q]).max()) for q in range(N_QUART))
    return tile_quarter, over_max


def _repair_quarters(M, tq, cfg, rng, iters=20000):
    """Local-search swap repair: drive per-(dst tile, quarter) counts
    under the per-quarter hard caps by swapping tiles between quarters."""
    caps = np.array([qb * P for qb in cfg.qbs], dtype=np.float64)
    n_tiles = cfg.n_tiles
    Mf = M.astype(np.float64)
    R = np.zeros((N_QUART, n_tiles), dtype=np.float64)
    for s in range(n_tiles):
        R[tq[s]] += Mf[s]
    members = [list(np.where(tq == q)[0]) for q in range(N_QUART)]

    def viol(q, r):
        o = np.maximum(r - caps[q], 0)
        return (o * o).sum(axis=-1)

    stall = 0
    for _ in range(iters):
        if all((R[q] <= caps[q]).all() for q in range(N_QUART)):
            return tq, 0.0
        q1 = int(np.argmax([viol(q, R[q]) for q in range(N_QUART)]))
        d = int(np.argmax(R[q1] - caps[q1]))
        mem1 = members[q1]
        contrib = M[mem1, d]
        ncand = 4 if stall < 80 else 10
        cand1 = [mem1[i] for i in np.argsort(-contrib)[:ncand]]
        base1 = float(viol(q1, R[q1]))
        best = None
        for s1 in cand1:
            r1_wo = R[q1] - Mf[s1]
            for q2 in range(N_QUART):
                if q2 == q1:
                    continue
                mem2 = np.asarray(members[q2])
                base2 = float(viol(q2, R[q2]))
                nr1 = r1_wo[None, :] + Mf[mem2]
                nr2 = (R[q2] + Mf[s1])[None, :] - Mf[mem2]
                v1 = np.maximum(nr1 - caps[q1], 0)
                v2 = np.maximum(nr2 - caps[q2], 0)
                delta = (v1 * v1).sum(1) + (v2 * v2).sum(1) - base1 - base2
                i = int(np.argmin(delta))
                if best is None or delta[i] < best[0]:
                    best = (float(delta[i]), s1, int(mem2[i]), q1, q2)
        if best is None or best[0] >= -1e-9:
            stall += 1
            if stall > 300:
                break
            q2 = int(rng.integers(0, N_QUART - 1))
            q2 = q2 if q2 < q1 else q2 + 1
            s1 = members[q1][int(rng.integers(len(members[q1])))]
            s2 = members[q2][int(rng.integers(len(members[q2])))]
            best = (0.0, s1, s2, q1, q2)
        else:
            stall = 0
        _, s1, s2, qa, qb_ = best
        R[qa] += Mf[s2] - Mf[s1]
        R[qb_] += Mf[s1] - Mf[s2]
        members[qa].remove(s1)
        members[qa].append(s2)
        members[qb_].remove(s2)
        members[qb_].append(s1)
        tq[s1], tq[s2] = qb_, qa
    if all((R[q] <= caps[q]).all() for q in range(N_QUART)):
        return tq, 0.0
    return tq, float(max(viol(q, R[q]) for q in range(N_QUART)))


def _pack_tiles_to_quarters(M, cfg):
    rng = np.random.default_rng(1234)
    attempts = [(np.argsort(-M.sum(axis=1), kind="stable"), 16),
                (np.argsort(-M.sum(axis=1), kind="stable"), 8),
                (rng.permutation(cfg.n_tiles), 16)]
    best_max = None
    for order, margin in attempts:
        tq, over = _try_pack_quarters(M, cfg, order, margin)
        if over <= 0:
            return tq
        tq, vmax = _repair_quarters(M, tq, cfg, rng)
        if vmax == 0:
            return tq
        if best_max is None or vmax < best_max:
            best_max = vmax
    raise RuntimeError(f"quarter packing failed: viol {best_max}")


def prepare(x, edge_index, cfg, fold=True):
    n = cfg.n_nodes
    src = np.asarray(edge_index[0], dtype=np.int64)
    dst = np.asarray(edge_index[1], dtype=np.int64)
    deg = (np.bincount(dst, minlength=n) + 1).astype(np.float64)
    dinv = 1.0 / np.sqrt(deg)

    node_tile, node_slot = _pack_nodes_to_tiles(deg, cfg)

    stile = node_tile[src]
    dtile = node_tile[dst]
    n_tiles = cfg.n_tiles
    M = np.zeros((n_tiles, n_tiles), dtype=np.int64)
    np.add.at(M, (stile, dtile), 1)

    tile_quarter = _pack_tiles_to_quarters(M, cfg)

    # quarter tiles -> (core, local quarter slot): balance per-core load
    tile_pos = np.empty(n_tiles, dtype=np.int64)
    tload = M.sum(axis=0)
    core_load = np.zeros(N_CORES, dtype=np.int64)
    for q in range(N_QUART):
        tiles_q = np.where(tile_quarter == q)[0]
        tiles_q = tiles_q[np.argsort(-tload[tiles_q], kind="stable")]
        fill = np.zeros(N_CORES, dtype=np.int64)
        for t in tiles_q:
            avail = np.where(fill < cfg.tpqs[q])[0]
            k = avail[np.argmin(core_load[avail])]
            tile_pos[t] = k * cfg.tpc + cfg.qbase[q] + fill[k]
            fill[k] += 1
            core_load[k] += tload[t]

    row_id = tile_pos[node_tile] * P + node_slot

    # quarter-table row of a node (node's slice concatenated per core)
    lrow = row_id % cfg.shard
    core_of = row_id // cfg.shard
    q_of = tile_quarter[node_tile]
    qbase_rows = np.asarray(cfg.qbase, dtype=np.int64) * P
    slice_rows = np.asarray(cfg.slice_rows_q, dtype=np.int64)
    qtab_row = core_of * slice_rows[q_of] + (lrow - qbase_rows[q_of])

    # ---- per-edge message placement (per source-quarter, ragged caps)
    drow_tile = tile_pos[node_tile[dst]]
    dslot = node_slot[dst]
    squart = q_of[src]
    srow_qt = qtab_row[src]

    m_idx_q, m_w_q, m_dslot_q = [], [], []
    for q in range(N_QUART):
        capq = cfg.qbs[q] * P
        mask = squart == q
        seg_q = drow_tile[mask]
        srow_q = srow_qt[mask]
        dslot_q = dslot[mask]
        # secondary sort by gather row for HBM locality within a segment
        order = np.lexsort((srow_q, seg_q))
        seg_s = seg_q[order]
        cnt = np.bincount(seg_s, minlength=n_tiles)
        if cnt.max() > capq:
            raise RuntimeError(f"segment overflow q{q}: "
                               f"{cnt.max()} > {capq}")
        start = np.concatenate([[0], np.cumsum(cnt)])
        within = np.arange(len(seg_s)) - start[seg_s]
        pslot = seg_s * capq + within
        mi = np.zeros(n_tiles * capq, dtype=np.int64)
        mw = np.zeros(n_tiles * capq, dtype=np.float32)
        md = np.zeros(n_tiles * capq, dtype=np.int64)
        # pad slots repeat the segment's last valid row (likely still in
        # the HBM row buffer) rather than hammering row 0
        pad_mask = np.ones(n_tiles * capq, dtype=bool)
        pad_mask[pslot] = False
        have = cnt > 0
        seg_last = np.zeros(n_tiles, dtype=np.int64)
        seg_last[have] = srow_q[order][
            np.minimum(start[1:][have] - 1, len(order) - 1)]
        src_fill = np.repeat(seg_last, capq)
        mi[pad_mask] = src_fill[pad_mask]
        mi[pslot] = srow_q[order]
        mw[pslot] = 1.0
        md[pslot] = dslot_q[order]
        m_idx_q.append(mi.reshape(n_tiles, cfg.qbs[q], P))
        m_w_q.append(mw.reshape(n_tiles, cfg.qbs[q], P))
        m_dslot_q.append(md.reshape(n_tiles, cfg.qbs[q], P))

    xTp = np.zeros((D, cfg.npad), dtype=np.float32)
    xv = np.asarray(x, dtype=np.float32)
    if fold:
        # uniform deferred-dinv form: layer tables scale by dinv^2, so
        # pre-divide the input (x has no deferred factor yet)
        xv = xv / dinv.astype(np.float32)[:, None]
    xTp[:, row_id] = xv.T

    dinv_row = np.zeros(cfg.npad, dtype=np.float32)
    dinv_row[row_id] = dinv.astype(np.float32)

    sdt = np.float16 if cfg.fp16 else np.float32
    per_core = []
    for k in range(N_CORES):
        # gather-block order: pass-major (q), then tile, qb (chunking
        # only groups instructions; the column layout is tile-major)
        gblocks = []
        for q in range(N_QUART):
            for ti in range(cfg.tpc):
                for qb_i in range(cfg.qbs[q]):
                    gblocks.append((k * cfg.tpc + ti, q, qb_i))
        blk_idx = np.stack([m_idx_q[q][pos, qb_i]
                            for (pos, q, qb_i) in gblocks])  # [ngblk, 128]
        blk_w = np.stack([m_w_q[q][pos, qb_i]
                          for (pos, q, qb_i) in gblocks])
        blk_dslot = np.stack([m_dslot_q[q][pos, qb_i]
                              for (pos, q, qb_i) in gblocks])

        midx = blk_idx.reshape(-1)
        ncols = cfg.ngmsg // 16
        idx16 = np.empty((16, ncols), dtype=np.int16)
        ar = np.arange(cfg.ngmsg)
        idx16[ar % 16, ar // 16] = midx.astype(np.int16)
        idx16 = np.tile(idx16, (8, 1))

        # S^T blocks: {0,1} one-hot gather blocks + one identity (fp8)
        ngblk = len(gblocks)
        sts = np.zeros((P, (ngblk + 1) * P), dtype=np.float32)
        bcol = (np.arange(ngblk)[:, None] * P + blk_dslot)
        prow = np.tile(np.arange(P)[None, :], (ngblk, 1))
        sts[prow.ravel(), bcol.ravel()] = blk_w.ravel()
        sts[np.arange(P), ngblk * P + np.arange(P)] = 1.0

        shard_cols = np.arange(k * cfg.shard, (k + 1) * cfg.shard)
        dinv_tile = dinv_row[shard_cols].reshape(cfg.tpc, P)  # [tpc, P]
        if fold:
            # dinv[dst] deferred through the next layer's LeakyReLU
            # (positively homogeneous): table scale becomes dinv^2
            dinv_tile = dinv_tile * dinv_tile
        # [P(slot), tpc*D]: column t*D+f holds dinv(slot-node of tile t)
        hdinv = np.repeat(dinv_tile[:, :, None], D, axis=2)  # [tpc,P,D]
        hdinv = hdinv.transpose(1, 0, 2).reshape(P, cfg.tpc * D)
        xdinv = np.tile(dinv_row[shard_cols][None, :], (P, 1))

        per_core.append({
            "xT": np.ascontiguousarray(
                xTp[:, k * cfg.shard:(k + 1) * cfg.shard]).astype(sdt),
            "idx16": idx16,
            "sts": sts.astype(ml_dtypes.float8_e4m3fn),
            "hdinv": np.ascontiguousarray(hdinv, dtype=sdt),
            "xdinv": xdinv.astype(sdt),
        })
    return per_core, row_id


# ------------------------------------------------------------ bass build

_FP = mybir.dt.float32


def build_program(cfg, ablate=(), repeats=1, nqueues=4, ag=4,
                  hpsb=3, opsb=3, mbufs=12, qopsb=2,
                  chunk=None, fold=True, st_scalar=True, actb=6,
                  selfdefer=True, cceng="gpsimd", ccinterleave=True,
                  spkt=True, scratch=16384, ccshift=1):
    """ablate: subset of {"noA","noB","noC","nogather"} for timing
    experiments (results wrong). repeats: timing instrument.

    cceng: engine hosting the AllGather collectives. The issuing engine
    queue BLOCKS until the collective completes, so they must not share
    a queue with phase C's gather desc-gen (gpsimd). The BIR verifier
    rejects collectives on SP/sync; ACT works and only carries async
    HWDGE issues (sts) + phase-A Lrelu. ccinterleave emits AG0 after
    slice 0's h DMAs and AG q>=1 at quarter q's phase-C head, so the
    collective chain runs back-to-back underneath phase C."""
    nc = bacc.Bacc("TRN2", target_bir_lowering=False, debug=False,
                   num_devices=N_CORES, num_swdge_queues=nqueues,
                   dynamic_dma_scratch_size=scratch)
    gq = [0]
    if chunk is not None:
        import copy as _copy
        cfg = _copy.copy(cfg)
        cfg.chunk = chunk
        cfg.chunks_q = tuple(chunk * max(cfg.qbs) // qb for qb in cfg.qbs)
    _HD = mybir.dt.float16 if cfg.fp16 else _FP
    xT_in = nc.declare_dram_parameter("xT", [D, cfg.shard], _HD,
                                      isOutput=False)
    idx_in = nc.declare_dram_parameter("idx16", [P, cfg.ngmsg // 16],
                                       mybir.dt.int16, isOutput=False)
    _F8 = mybir.dt.float8e4
    nst = cfg.ngmsg + P
    sts_in = nc.declare_dram_parameter("sts", [P, nst], _F8,
                                       isOutput=False)
    dinv_in = nc.declare_dram_parameter("hdinv", [P, cfg.tpc * D], _HD,
                                        isOutput=False)
    xdinv_in = nc.declare_dram_parameter("xdinv", [P, cfg.shard], _HD,
                                         isOutput=False)
    w_ins = [nc.declare_dram_parameter(f"W{i}", [D, D], _FP, isOutput=False)
             for i in range(3)]
    b_ins = [nc.declare_dram_parameter(f"b{i}", [D, 1], _FP, isOutput=False)
             for i in range(3)]
    out_dram = nc.declare_dram_parameter("out", [D, cfg.shard], _HD,
                                         isOutput=True)

    h_slices = [nc.dram_tensor(f"h_sl{q}", [cfg.slice_rows_q[q], D], _HD)
                for q in range(N_QUART)]
    tables = [nc.dram_tensor(f"table{q}", [cfg.qrows_q[q], D], _HD,
                             addr_space="Shared")
              for q in range(N_QUART)]
    if "agfree" in ablate:
        # timing probe: gathers read an unrelated DRAM tensor so they have
        # no data dependency on the AllGathers (results wrong)
        gtables = [nc.dram_tensor(f"dtable{q}", [cfg.qrows_q[q], D], _HD)
                   for q in range(N_QUART)]
    else:
        gtables = tables

    groups = [list(range(N_CORES))]

    # msg cols per (chunk, quarter) — same for every quarter by design
    gcols = cfg.chunk * max(cfg.qbs) * P

    with tile.TileContext(nc, num_cores=N_CORES) as tc:
        with (
            tc.tile_pool(name="const", bufs=1) as cpool,
            tc.tile_pool(name="actp", bufs=actb) as actp,
            tc.tile_pool(name="hps", bufs=hpsb, space="PSUM") as hps,
            tc.tile_pool(name="msgp", bufs=mbufs) as msgp,
            tc.tile_pool(name="stp", bufs=mbufs) as stp,
            tc.tile_pool(name="ops", bufs=opsb, space="PSUM") as ops,
            tc.tile_pool(name="qops", bufs=qopsb, space="PSUM") as qops,
        ):
            xa = cpool.tile([D, cfg.shard], _HD, tag="xa")
            xb = cpool.tile([D, cfg.shard], _HD, tag="xb")
            idxt = cpool.tile([P, cfg.ngmsg // 16], mybir.dt.int16,
                              tag="idxt")
            wts = [cpool.tile([D, D], _FP, tag=f"w{i}", name=f"w{i}")
                   for i in range(3)]
            bts = [cpool.tile([D, 1], _FP, tag=f"b{i}", name=f"b{i}")
                   for i in range(3)]
            # persistent on-chip h shard (node-major per tile), identity
            # self block, per-slot dinv, per-column dinv broadcast
            hsall = cpool.tile([P, cfg.tpc * D], _HD, tag="hsall")
            ident = cpool.tile([P, P], _F8, tag="ident")
            hdinv = cpool.tile([P, cfg.tpc * D], _HD, tag="hdinv")
            xdinv = cpool.tile([P, cfg.shard], _HD, tag="xdinv")

            nc.sync.dma_start(out=xa[:], in_=xT_in[:])
            nc.sync.dma_start(out=idxt[:], in_=idx_in[:])
            nc.sync.dma_start(out=ident[:],
                              in_=sts_in[:, cfg.ngmsg:cfg.ngmsg + P])
            nc.sync.dma_start(out=hdinv[:], in_=dinv_in[:])
            nc.sync.dma_start(out=xdinv[:], in_=xdinv_in[:])
            for i in range(3):
                nc.sync.dma_start(out=wts[i][:], in_=w_ins[i][:])
                nc.sync.dma_start(out=bts[i][:], in_=b_ins[i][:])

            # phase-A groups of <=8 tiles, never crossing a slice boundary
            agroups = []
            for sl in range(N_QUART):
                g0 = 0
                while g0 < cfg.tpqs[sl]:
                    n = min(ag, cfg.tpqs[sl] - g0)
                    agroups.append((cfg.qbase[sl] + g0, n))
                    g0 += n

            xbufs = [xa, xb]
            cc_eng = {"sync": nc.sync, "gpsimd": nc.gpsimd,
                      "scalar": nc.scalar}[cceng]

            ag_emitted = set()

            def emit_ag(q):
                if q in ag_emitted:
                    return
                ag_emitted.add(q)
                # collective_compute is defined on BassGpSimd but only uses
                # generic BassEngine facilities — unbound call hosts the
                # instruction on cc_eng's queue instead of gpsimd's
                type(nc.gpsimd).collective_compute(
                    cc_eng, "AllGather", mybir.AluOpType.bypass,
                    ins=[h_slices[q][:]], outs=[tables[q][:]],
                    replica_groups=groups)

            for layer3 in range(3 * repeats):
                layer = layer3 % 3
                ag_emitted.clear()
                xcur = xbufs[layer % 2]
                xnxt = xbufs[(layer + 1) % 2]
                # ---- phase A: h = Lrelu(x)^T W in groups; self pass fused
                with nc.named_scope(f"L{layer}_matmul"):
                    if "noA" in ablate:
                        nc.gpsimd.memset(xnxt[:], 0.0)
                    for (t0, n) in (agroups if "noA" not in ablate else []):
                        act = actp.tile([D, ag * P], _FP, tag="act")
                        nc.scalar.activation(
                            act[:, :n * P], xcur[:, t0 * P:(t0 + n) * P],
                            mybir.ActivationFunctionType.Lrelu, alpha=0.01)
                        hp = hps.tile([P, ag * D], _FP, tag="hp")
                        for g in range(n):
                            nc.tensor.matmul(hp[:, g * D:(g + 1) * D],
                                             lhsT=act[:, g * P:(g + 1) * P],
                                             rhs=wts[layer][:],
                                             start=True, stop=True)
                        nc.vector.tensor_tensor(
                            out=hsall[:, t0 * D:(t0 + n) * D],
                            in0=hp[:, :n * D],
                            in1=hdinv[:, t0 * D:(t0 + n) * D],
                            op=mybir.AluOpType.mult)
                        sl = next(s for s in range(N_QUART)
                                  if cfg.qbase[s] <= t0
                                  < cfg.qbase[s] + cfg.tpqs[s])
                        lt = t0 - cfg.qbase[sl]
                        nc.sync.dma_start(
                            out=h_slices[sl][lt * P:(lt + n) * P, :]
                            .rearrange("(b p) e -> p b e", p=P),
                            in_=hsall[:, t0 * D:(t0 + n) * D]
                            .rearrange("p (b e) -> p b e", e=D))
                        if (ccinterleave and "noB" not in ablate
                                and sl == 0 and lt + n == cfg.tpqs[0]):
                            emit_ag(0)
                        if not selfdefer:
                            op = ops.tile([D, ag * P], _FP, tag="op")
                            for g in range(n):
                                t = t0 + g
                                nc.tensor.matmul(
                                    op[:, g * P:(g + 1) * P],
                                    lhsT=hsall[:, t * D:(t + 1) * D],
                                    rhs=ident[:],
                                    start=True, stop=True)
                            nc.vector.tensor_copy(
                                out=xnxt[:, t0 * P:(t0 + n) * P],
                                in_=op[:, :n * P])
                    for (t0, n) in (agroups if selfdefer
                                    and "noA" not in ablate else []):
                        op = ops.tile([D, ag * P], _FP, tag="op")
                        for g in range(n):
                            t = t0 + g
                            nc.tensor.matmul(
                                op[:, g * P:(g + 1) * P],
                                lhsT=hsall[:, t * D:(t + 1) * D],
                                rhs=ident[:],
                                start=True, stop=True)
                        nc.vector.tensor_copy(
                            out=xnxt[:, t0 * P:(t0 + n) * P],
                            in_=op[:, :n * P])
                # ---- phase B: any AllGathers not already interleaved
                # into phase A (AG0) or phase C (AG1-3)
                with nc.named_scope(f"L{layer}_allgather"):
                    if "noB" not in ablate and (
                            not ccinterleave or "noA" in ablate
                            or "noC" in ablate):
                        for q in range(N_QUART):
                            emit_ag(q)
                # ---- phase C: 4 quarter passes (self fused into phase A)
                with nc.named_scope(f"L{layer}_aggregate"):
                    if "noC" in ablate:
                        pass
                    else:
                        # quarter passes
                        for q in range(N_QUART):
                            if ccinterleave and "noB" not in ablate:
                                # ccshift=1: dispatch the NEXT quarter's
                                # AG at this quarter's head so its issue
                                # precedes the gathers running under it
                                # (the pool engine queue is 8-deep strict
                                # FIFO — an AG emitted after a quarter's
                                # desc-gens dispatches ~20us late).
                                # ccshift=2: dispatch ALL remaining AGs
                                # at phase C's start.
                                if ccshift == 2:
                                    if q == 0:
                                        for qq in range(1, N_QUART):
                                            emit_ag(qq)
                                elif ccshift == 1:
                                    if q + 1 < N_QUART:
                                        emit_ag(q + 1)
                                    if q > 0:
                                        emit_ag(q)  # no-op unless skipped
                                elif q > 0:
                                    emit_ag(q)
                            qb_q = cfg.qbs[q]
                            for (t0, nt) in cfg.chunk_list(q):
                                ncol = nt * qb_q * P
                                c0 = cfg.qmsg0[q] + t0 * qb_q * P
                                msg = msgp.tile([P, gcols], _HD,
                                                tag="gmsg")
                                if True:
                                    parts = [(0, ncol)]
                                    if "nogather" in ablate:
                                        # minimal write to satisfy the tile
                                        # tracker; removes ~95% of gather
                                        parts = [(0, 128)]
                                    for (h0, h1) in parts:
                                        if h1 <= h0:
                                            continue
                                        nc.gpsimd.dma_gather(
                                            out_ap=msg[:, h0:h1].rearrange(
                                                "p (b e) -> p b e", e=P),
                                            in_ap=gtables[q][:],
                                            idxs_ap=idxt[
                                                :, (c0 + h0) // 16:
                                                (c0 + h1) // 16],
                                            num_idxs=h1 - h0,
                                            num_idxs_reg=h1 - h0,
                                            elem_size=P,
                                            queue_num=gq[0] % nqueues,
                                            single_packet=spkt and (h1 - h0) <= 1024,
                                        )
                                        gq[0] += 1
                                st = stp.tile([P, gcols], _F8, tag="gst")
                                steng = (nc.scalar if st_scalar
                                         else nc.sync)
                                steng.dma_start(
                                    out=st[:, :ncol],
                                    in_=sts_in[:, c0:c0 + ncol])
                                for ti0 in range(0, nt, 4):
                                    ns = min(4, nt - ti0)
                                    qop = qops.tile([D, 4 * P], _FP,
                                                    tag="qop")
                                    for ti in range(ti0, ti0 + ns):
                                        for qb_i in range(qb_q):
                                            col = (ti * qb_q + qb_i) * P
                                            nc.tensor.matmul(
                                                qop[:, (ti - ti0) * P:
                                                    (ti - ti0 + 1) * P],
                                                lhsT=msg[:, col:col + P],
                                                rhs=st[:, col:col + P],
                                                start=(qb_i == 0),
                                                stop=(qb_i == qb_q - 1))
                                    c0s = (t0 + ti0) * P
                                    nc.vector.tensor_add(
                                        out=xnxt[:, c0s:c0s + ns * P],
                                        in0=xnxt[:, c0s:c0s + ns * P],
                                        in1=qop[:, :ns * P])
                                    if q == N_QUART - 1 and (
                                            not fold or layer == 2):
                                        nc.vector.tensor_tensor(
                                            out=xnxt[:, c0s:c0s + ns * P],
                                            in0=xnxt[:, c0s:c0s + ns * P],
                                            in1=xdinv[:, c0s:c0s + ns * P],
                                            op=mybir.AluOpType.mult)
                                        nc.vector.tensor_scalar_add(
                                            out=xnxt[:, c0s:c0s + ns * P],
                                            in0=xnxt[:, c0s:c0s + ns * P],
                                            scalar1=bts[layer][:])
                if layer == 2:
                    nc.sync.dma_start(out=out_dram[:], in_=xnxt[:])
    nc.compile()
    return nc


_PROGRAM_CACHE = {}


def _get_program(cfg, fold=True):
    key = (cfg.n_nodes, cfg.n_edges, cfg.tpc, cfg.qbs, cfg.qsizes,
           cfg.chunk, cfg.fp16, fold)
    if key not in _PROGRAM_CACHE:
        _PROGRAM_CACHE[key] = build_program(cfg, fold=fold)
    return _PROGRAM_CACHE[key]


# --------------------------------------------------------------- driver


def run(x, edge_index, W1, b1, W2, b2, W3, b3, cfg, trace=False,
        trace_kwargs=None):
    fold = not (np.any(np.asarray(b1)) or np.any(np.asarray(b2))
                or np.any(np.asarray(b3)))
    try:
        per_core, row_id = prepare(x, edge_index, cfg, fold=fold)
    except RuntimeError:
        # uneven-quarter packing infeasible for this graph: fall back to
        # uniform quarters (qb=2 everywhere, 28% slot headroom)
        uni = cfg.n_tiles // N_QUART
        cfg = Cfg(cfg.n_nodes, cfg.n_edges, cfg.tpc, cfg.chunk,
                  qsizes=(uni,) * N_QUART, qbs=(2, 2, 2, 2),
                  fp16=cfg.fp16)
        per_core, row_id = prepare(x, edge_index, cfg, fold=fold)
    nc = _get_program(cfg, fold=fold)
    ws = [np.asarray(a, dtype=np.float32) for a in (W1, W2, W3)]
    bs = [np.asarray(a, dtype=np.float32).reshape(D, 1) for a in (b1, b2, b3)]
    in_maps = []
    for k in range(N_CORES):
        m = dict(per_core[k])
        for i in range(3):
            m[f"W{i}"] = ws[i]
            m[f"b{i}"] = bs[i]
        in_maps.append(m)
    res = run_bass_kernel_spmd(nc, in_maps, list(range(N_CORES)),
                               trace=trace, **(trace_kwargs or {}))
    outT = np.concatenate([res.results[k]["out"] for k in range(N_CORES)],
                          axis=1)
    out = np.empty((cfg.n_nodes, D), dtype=np.float32)
    out[:, :] = outT[:, row_id].T
    return out, res


def kernel(x, edge_index, W1, b1, W2, b2, W3, b3):
    out, _ = run(x, edge_index, W1, b1, W2, b2, W3, b3, FULL)
    return out

